# revision 52
# baseline (speedup 1.0000x reference)
"""Multi-head attention kernel for 8 TRN2 NeuronCores.

Reference: out = einsum('dha,blha->bld', O, softmax(q k^T) v) with
q/k/v = einsum('dha,bld->blha', W, x);  B=4, L=2048, D=1024, H=16, A=64.

Sharding: core c handles batch b = c//2 and head-group hg = c%2 (8 heads,
data parallel on B x tensor parallel on heads). Each core computes a partial
output [L, D] summed over its 8 heads; the host adds the two head-group
partials per batch.

Schedule: both engines are near-saturated -- ACT does 256 [128,1024] exp
ACTIVATEs (~285us) and the PE ~350us of matmul (projections 55us + v 28 +
scores 55 + ctx 110 + output 28 + weight loads).  The whole kernel is one
software pipeline over head pairs so neither engine ever waits long:
  - all attention sections (scores -> exp -> ctx -> normalize) are issued
    inside tc.high_priority blocks; projections, the natural-layout v
    pass, and the output projection ride at natural priority, so the Tile
    scheduler runs them only in PE slack.  There is no serial projection
    phase and only ~half the output projection trails the last exp.
  - scores for the two heads of a pair run concurrently on the two PE row
    halves (K=64 row tiling via base_partition 0/64), writing the two
    bank-halves of one [128,1024] psum tile; one exp ACTIVATE covers both.
  - v is projected directly to natural [lk, head*64] layout (x chunk
    stationary, V weights moving), in two head-halves so pairs 0/1's v is
    ready early; a ones column makes softmax denominators fall out of the
    ctx matmul.  No PE transposes.
  - per (pair, strip, sub): ctx accumulates in 2 psum banks; normalize =
    SBUF->SBUF partition-scatter DMA (the 1-lane DVE reciprocal is
    ~6.4ns/elem, so spread it over 64 lanes) + reciprocal + DRAM-bounce
    partition broadcast + DVE multiply, hidden under the next section.
  - pair 3 processes strip 1 before strip 0, so outproj(strip 1) overlaps
    pair 3's strip-0 sections; the tail outproj evacuates PSUM through the
    then-idle ACT engine and borrows the freed scores psum banks.
  - input DMAs ride one strictly criticality-ordered queue (queues share
    ~360GB/s fairly, so more queues only dilute the critical prefix);
    dummy matmuls warm the PE HAM clock gate during the DMA wait.
PSUM: scores staging 2x2 banks + ctx accumulators 2 + proj/outproj scratch 2.

Measured on TRN2 (neuron-profile): 390-402 us exec at the 2.4GHz power
state (~462 us when the chip sits throttled at 2.0GHz; all engine work
scales 1.2x there), rel err 8.0e-3.  PE occupancy is ~94% between its
first and last matmul (348.8us busy, 21.6us idle); the scalar engine runs
~285us of exp.
"""

import sys

sys.path.insert(0, "/opt/trn_rl_repo")

from contextlib import ExitStack

import numpy as np
import ml_dtypes

import concourse.bass as bass  # noqa: F401
import concourse.tile as tile
from concourse import bacc, mybir
from concourse.bass_utils import run_bass_kernel_spmd

B, L, D, H, A = 4, 2048, 1024, 16, 64
HC = 8          # heads per core
NP = HC // 2    # head pairs per core
DC = D // 128   # d chunks
LC = L // 128   # lk chunks
HA = HC * A     # 512
NW = 4          # 512-wide lq windows

f32 = mybir.dt.float32
bf16 = mybir.dt.bfloat16
f16 = mybir.dt.float16
ExpF = mybir.ActivationFunctionType.Exp


def build_graph():
    nc = bacc.Bacc("TRN2", target_bir_lowering=False, debug=False, num_devices=8)
    # note: partition-interleaved gathers (1KB runs) sustain ~345GB/s; long
    # per-partition contiguous runs drop to ~60GB/s (single SBUF write port)
    xqT_e = nc.dram_tensor("xqT", [D, L], f16, kind="ExternalInput").ap()
    xkvT_e = nc.dram_tensor("xkvT", [D, L], f16, kind="ExternalInput").ap()
    Qw_e = nc.dram_tensor("Qw", [D, HA], f16, kind="ExternalInput").ap()
    Kw_e = nc.dram_tensor("Kw", [D, HA], f16, kind="ExternalInput").ap()
    Vw_e = nc.dram_tensor("Vw", [D, HA], f16, kind="ExternalInput").ap()
    OwT_e = nc.dram_tensor("OwT", [HA, D], bf16, kind="ExternalInput").ap()
    out_e = nc.dram_tensor("out", [L, D], f32, kind="ExternalOutput").ap()

    with tile.TileContext(nc) as tc, ExitStack() as ctx:
        pers = ctx.enter_context(tc.tile_pool(name="pers", bufs=1))
        sb = ctx.enter_context(tc.tile_pool(name="sb", bufs=1))
        drp = ctx.enter_context(tc.tile_pool(name="drp", bufs=2, space="DRAM"))
        pp = ctx.enter_context(tc.tile_pool(name="pp", bufs=1, space="PSUM"))

        # ---- resident tiles ----
        xkv_s = pers.tile([128, DC, L], f16, tag="xkv", name="xkv_s")
        Qw_s = pers.tile([128, DC, HA], f16, tag="Qw", name="Qw_s")
        Kw_s = pers.tile([128, DC, HA], f16, tag="Kw", name="Kw_s")
        Vw_s = pers.tile([128, DC, HA], f16, tag="Vw", name="Vw_s")
        ow = [pers.tile([128, D], bf16, tag=f"ow{p}", name=f"ow{p}") for p in range(NP)]
        # vaug[lk part, lk chunk, head, 0:64 v | 64 ones | 65 pad]
        vaug = pers.tile([128, LC, HC, 66], bf16, tag="vaug", name="vaug")
        ctp = [[pers.tile([128, 1024], bf16, tag=f"ctp{s}_{p}", name=f"ctp{s}_{p}")
                for p in range(NP)] for s in range(2)]
        warm_in = pers.tile([1, 16], f32, tag="warm_in", name="warm_in")
        warm = pers.tile([1, 16], f32, tag="warm", name="warm")

        def wap(w_e):
            # [128 part, DC d-chunks, HA cols] gather of W[d*128+p, c]
            # (1KB descriptor runs; narrow column slices would be
            # descriptor-rate bound at ~25GB/s, 1KB runs hit ~345GB/s)
            return bass.AP(
                tensor=w_e.tensor,
                offset=w_e.offset,
                ap=[[HA, 128], [128 * HA, DC], [1, HA]],
            )

        def xwin_ap(x_e, w):
            return bass.AP(
                tensor=x_e.tensor,
                offset=x_e.offset + w * 512,
                ap=[[L, 128], [128 * L, DC], [1, 512]],
            )

        def wap_d(w_e, d0, d1):
            return bass.AP(
                tensor=w_e.tensor,
                offset=w_e.offset + d0 * 128 * HA,
                ap=[[HA, 128], [128 * HA, d1 - d0], [1, HA]],
            )

        def xwin_d_ap(x_e, w, d0, d1):
            return bass.AP(
                tensor=x_e.tensor,
                offset=x_e.offset + w * 512 + d0 * 128 * L,
                ap=[[L, 128], [128 * L, d1 - d0], [1, 512]],
            )

        # ---- input DMAs: one strictly criticality-ordered queue (DMA
        # queues share the ~360GB/s per-core bandwidth fairly, so extra
        # queues only dilute the critical prefix; a single queue saturates).
        # The first window's K/x/Q slices are interleaved per d-pair so the
        # projection d-loops start on the first 512KB instead of the full
        # 3MB (region-level tile deps admit partial-tile consumption).
        for d in range(0, DC, 2):
            nc.sync.dma_start(out=Kw_s[:, d:d + 2, :], in_=wap_d(Kw_e, d, d + 2))
            nc.sync.dma_start(out=xkv_s[:, d:d + 2, 0:512],
                              in_=xwin_d_ap(xkvT_e, 0, d, d + 2))
            nc.sync.dma_start(out=Qw_s[:, d:d + 2, :], in_=wap_d(Qw_e, d, d + 2))
        nc.sync.dma_start(out=Vw_s[:], in_=wap(Vw_e))
        for w in range(1, NW):
            nc.sync.dma_start(out=xkv_s[:, :, w * 512:(w + 1) * 512],
                              in_=xwin_ap(xkvT_e, w))
        for p in range(NP):
            nc.sync.dma_start(out=ow[p][:], in_=OwT_e[p * 128:(p + 1) * 128, :])

        # warm the ACT exp table during the ramp
        nc.vector.memset(warm_in[:], 0.0)
        nc.scalar.activation(warm[:], warm_in[:], ExpF)
        nc.vector.memset(vaug[:, :, :, 64:65], 1.0)
        # warm the PE (HAM clock gate) with dummy matmuls during the input
        # DMA wait, so the first real projections run at 2.4GHz not 1.2GHz
        wmm = pers.tile([128, 512], bf16, tag="wmm", name="wmm")
        nc.vector.memset(wmm[:], 0.0)
        for i in range(12):
            wps = pp.tile([128, 512], f32, tag="scr", bufs=2, name=f"wps{i}")
            nc.tensor.matmul(wps[:], lhsT=wmm[:, 0:128], rhs=wmm[:],
                             start=True, stop=True)

        # qT/kT pair tiles: rows 0:64 head even, 64:128 head odd; 2 live pairs
        qTs = [sb.tile([128, L], bf16, tag="qT", bufs=2, name=f"qT{p}")
               for p in range(NP)]
        kTs = [sb.tile([128, L], bf16, tag="kT", bufs=2, name=f"kT{p}")
               for p in range(NP)]

        # ---- slack-priority producers ----
        def proj_window(which, p, w):
            """one 512-lq window of the q/k projection for pair p."""
            if which == "q":
                xt = sb.tile([128, DC, 512], f16, tag="xq", bufs=2,
                             name=f"xq_{p}_{w}")
                nc.gpsimd.dma_start(out=xt[:], in_=xwin_ap(xqT_e, w))
                rhs = lambda d: xt[:, d, :]
                ws, dst = Qw_s, qTs[p]
            else:
                rhs = lambda d: xkv_s[:, d, w * 512:(w + 1) * 512]
                ws, dst = Kw_s, kTs[p]
            pj = pp.tile([128, 512], f32, tag="scr", bufs=2,
                         name=f"pj_{which}_{p}_{w}")
            for d in range(DC):
                nc.tensor.matmul(pj[:], lhsT=ws[:, d, p * 128:(p + 1) * 128],
                                 rhs=rhs(d), start=(d == 0), stop=(d == DC - 1))
            nc.vector.tensor_copy(dst[:, w * 512:(w + 1) * 512], pj[:])

        def vnat_chunk(c, half):
            """natural-layout v for lk chunk c, heads half*4..half*4+3."""
            vn = pp.tile([128, 256], f32, tag="scr", bufs=2, name=f"vn_{c}_{half}")
            for d in range(DC):
                nc.tensor.matmul(
                    vn[:], lhsT=xkv_s[:, d, c * 128:(c + 1) * 128],
                    rhs=Vw_s[:, d, half * 256:(half + 1) * 256],
                    start=(d == 0), stop=(d == DC - 1))
            nc.vector.tensor_copy(vaug[:, c, half * 4:(half + 1) * 4, 0:64], vn[:])

        # ---- high-priority attention pipeline ----
        def normalize(p, s, sub, pcs):
            stg = sb.tile([65, 1024], f32, tag="stg", bufs=2, name=f"stg{p}{s}{sub}")
            un = [sb.tile([64, 512], bf16, tag="un", bufs=4,
                          name=f"un{p}{s}{sub}{h2}") for h2 in range(2)]
            for h2 in range(2):
                nc.vector.tensor_copy(stg[64:65, h2 * 512:(h2 + 1) * 512],
                                      pcs[h2][64:65, :])
            for h2 in range(2):
                nc.vector.tensor_copy(un[h2][:], pcs[h2][0:64, :])
            # reciprocal is ~6.4ns/elem/lane on DVE: scatter the 1-partition
            # denominator row over 64 lanes first (SBUF->SBUF DMA)
            rsh = sb.tile([64, 16], f32, tag="rsh", bufs=2, name=f"rsh{p}{s}{sub}")
            nc.sync.dma_start(out=rsh[:], in_=stg[64:65, :])
            rrec = sb.tile([64, 16], f32, tag="rsh", bufs=2, name=f"rrec{p}{s}{sub}")
            nc.vector.reciprocal(rrec[:], rsh[:])
            dr2 = drp.tile([1, 1024], f32, tag="dr", bufs=2, name=f"dr2{p}{s}{sub}")
            nc.sync.dma_start(
                out=bass.AP(tensor=dr2.tensor, offset=dr2.offset, ap=[[16, 64], [1, 16]]),
                in_=rrec[:])
            cto = sb.tile([64, 512], bf16, tag="cto", bufs=2, name=f"cto{p}{s}{sub}")
            for h2 in range(2):
                pbs = sb.tile([64, 512], f32, tag="pbs", bufs=4,
                              name=f"pbs{p}{s}{sub}{h2}")
                bcast = bass.AP(tensor=dr2.tensor, offset=dr2.offset + h2 * 512,
                                ap=[[0, 64], [1, 512]])
                nc.sync.dma_start(out=pbs[:], in_=bcast)
                dst = (ctp[s][p][0:64, sub * 512:(sub + 1) * 512]
                       if h2 == 0 else cto[:])
                nc.vector.tensor_mul(dst, un[h2][:], pbs[:])
            nc.sync.dma_start(out=ctp[s][p][64:128, sub * 512:(sub + 1) * 512],
                              in_=cto[:])

        def attention(p, s, sub):
            qT, kT = qTs[p], kTs[p]
            w = s * 2 + sub
            pcs = [pp.tile([65, 512], f32, tag="pcs", bufs=2,
                           name=f"pc{p}{s}{sub}{h2}") for h2 in range(2)]
            for c in range(LC):
                sts = pp.tile([128, 1024], f32, tag="sts", bufs=2,
                              name=f"st{p}{s}{sub}{c}")
                for h2 in range(2):
                    nc.tensor.matmul(
                        sts[:, h2 * 512:(h2 + 1) * 512],
                        lhsT=kT[h2 * 64:(h2 + 1) * 64, c * 128:(c + 1) * 128],
                        rhs=qT[h2 * 64:(h2 + 1) * 64, w * 512:(w + 1) * 512],
                        start=True, stop=True)
                et = sb.tile([128, 1024], bf16, tag="exp", bufs=10,
                             name=f"et{p}{s}{sub}{c}")
                nc.scalar.activation(et[:], sts[:], ExpF)
                for h2 in range(2):
                    nc.tensor.matmul(pcs[h2][:],
                                     lhsT=vaug[:, c, 2 * p + h2, 0:65],
                                     rhs=et[:, h2 * 512:(h2 + 1) * 512],
                                     start=(c == 0), stop=(c == LC - 1))
            normalize(p, s, sub, pcs)

        def outproj(s, blocks, copy_eng="vector", tags=("scr",)):
            """output projection; copy_eng="scalar" routes the PSUM
            evacuation through the ACT engine -- only for the tail, after
            the last exp (when the sts banks are also free for po tiles)."""
            for gi, b in enumerate(blocks):
                for dt_ in range(2):
                    tg = tags[(2 * gi + dt_) % len(tags)]
                    po = pp.tile([128, 512], f32, tag=tg, bufs=2,
                                 name=f"po{s}_{b}_{dt_}")
                    for p in range(NP):
                        nc.tensor.matmul(
                            po[:], lhsT=ctp[s][p][:, b * 128:(b + 1) * 128],
                            rhs=ow[p][:, dt_ * 512:(dt_ + 1) * 512],
                            start=(p == 0), stop=(p == NP - 1))
                    ost = sb.tile([128, 512], f32, tag=f"ost{copy_eng}", bufs=3,
                                  name=f"ost{s}_{b}_{dt_}")
                    if copy_eng == "scalar" and (2 * gi + dt_) % 2 == 0:
                        # tail batch: both ACT and DVE are idle; alternate
                        nc.scalar.activation(ost[:], po[:],
                                             mybir.ActivationFunctionType.Copy)
                    else:
                        nc.vector.tensor_copy(ost[:], po[:])
                    row = s * 1024 + b * 128
                    dma_eng = (nc.gpsimd if copy_eng == "scalar"
                               and (2 * gi + dt_) % 2 == 1 else nc.sync)
                    dma_eng.dma_start(
                        out=out_e[row:row + 128, dt_ * 512:(dt_ + 1) * 512],
                        in_=ost[:])

        # ---- issue: slack producers at natural priority, deadline-ordered ----
        proj_window("k", 0, 0)
        proj_window("q", 0, 0)
        proj_window("k", 0, 1)
        proj_window("k", 0, 2)
        proj_window("k", 0, 3)
        for c in range(8):
            vnat_chunk(c, 0)
        proj_window("q", 0, 1)
        for c in range(8, LC):
            vnat_chunk(c, 0)
        proj_window("q", 0, 2)
        proj_window("q", 0, 3)
        for w in range(NW):
            proj_window("k", 1, w)
        proj_window("q", 1, 0)
        proj_window("q", 1, 1)
        for c in range(8):
            vnat_chunk(c, 1)
        proj_window("q", 1, 2)
        proj_window("q", 1, 3)
        for c in range(8, LC):
            vnat_chunk(c, 1)
        for w in range(NW):
            proj_window("k", 2, w)
        for w in range(NW):
            proj_window("q", 2, w)
        for w in range(NW):
            proj_window("k", 3, w)
        # pair 3 runs strip 1 first: q windows 2,3 are needed before 0,1
        for w in (2, 3, 0, 1):
            proj_window("q", 3, w)

        # ---- attention sections, strictly prioritized; outproj rides slack ----
        with tc.high_priority(offset=10 ** 6):
            for p in range(NP - 1):
                for s in range(2):
                    for sub in range(2):
                        attention(p, s, sub)
            attention(NP - 1, 1, 0)
            attention(NP - 1, 1, 1)
        outproj(1, list(range(8)))
        with tc.high_priority(offset=10 ** 6):
            attention(NP - 1, 0, 0)
        outproj(0, [0, 1, 2, 3])
        with tc.high_priority(offset=10 ** 6):
            attention(NP - 1, 0, 1)
        outproj(0, [4, 5, 6, 7], copy_eng="scalar", tags=("scr", "sts"))
    nc.compile()
    return nc


_NC = None


def _get_nc():
    global _NC
    if _NC is None:
        _NC = build_graph()
    return _NC


# test harness can override, e.g. {"trace": True}
RUN_KWARGS: dict = {}
LAST_RESULTS = None


def make_in_maps(xq, xkv, Q, K, V, O):
    xq = np.asarray(xq, np.float32)
    xkv = np.asarray(xkv, np.float32)
    Q = np.asarray(Q, np.float32)
    K = np.asarray(K, np.float32)
    V = np.asarray(V, np.float32)
    O = np.asarray(O, np.float32)
    # cores 2b and 2b+1 share batch b's transposed activations; compute once
    xqT_c = [np.ascontiguousarray(xq[b].T).astype(np.float16) for b in range(B)]
    xkvT_c = [np.ascontiguousarray(xkv[b].T).astype(np.float16) for b in range(B)]
    in_maps = []
    for core in range(8):
        b, hg = divmod(core, 2)
        hs = slice(hg * HC, (hg + 1) * HC)
        in_maps.append({
            "xqT": xqT_c[b],
            "xkvT": xkvT_c[b],
            "Qw": np.ascontiguousarray(Q[:, hs, :].reshape(D, HA)).astype(np.float16),
            "Kw": np.ascontiguousarray(K[:, hs, :].reshape(D, HA)).astype(np.float16),
            "Vw": np.ascontiguousarray(V[:, hs, :].reshape(D, HA)).astype(np.float16),
            "OwT": np.ascontiguousarray(
                O[:, hs, :].reshape(D, HA).T).astype(ml_dtypes.bfloat16),
        })
    return in_maps


def kernel(xq, xkv, Q, K, V, O):
    global LAST_RESULTS
    nc = _get_nc()
    in_maps = make_in_maps(xq, xkv, Q, K, V, O)
    res = run_bass_kernel_spmd(nc, in_maps, core_ids=list(range(8)), **RUN_KWARGS)
    LAST_RESULTS = res
    outs = [np.asarray(res.results[c]["out"], np.float32) for c in range(8)]
    return np.stack([outs[2 * b] + outs[2 * b + 1] for b in range(B)], axis=0)
